# revision 1
# baseline (speedup 1.0000x reference)
"""Multi-head attention (RMSNorm-QK + RoPE) Trainium2 Bass kernel.

Sharding: 8 cores = 4 batches x 2 head-groups (6 heads each).
Each core computes, for its (batch, head-group):
  qkv = x @ Wqkv_slice (+bias), rmsnorm+rope on q/k, softmax(q k^T/8) v,
  y_partial = attn_out @ Wproj_rows.
Host sums the two partial y's per batch and adds proj bias.

Matmuls run in float32r (fp32 storage, ~11-bit mantissa, full PE speed).
Inputs are pre-rounded to f32r on the host (RNE-11, verified vs device).
"""

import sys

for _p in ("/opt/trn_rl_repo", "/root/.axon_site/_ro/trn_rl_repo"):
    if _p not in sys.path:
        sys.path.insert(0, _p)

import numpy as np

import bass_rust
import concourse.bass as bass
import concourse.mybir as mybir
import concourse.tile as tile
from concourse.bass_utils import run_bass_kernel_spmd
from concourse.masks import make_identity

# Problem constants (hardcoded per contract)
B, N, D = 4, 2048, 768
H, HD = 12, 64
HPC = 6              # heads per core
NT = N // 128        # 16 seq tiles
EPS = 1e-6
THETA = 10000.0
SCALE = HD ** -0.5   # 0.125

F32 = mybir.dt.float32
F32R = mybir.dt.float32r

KERNEL_TRACE = False
_CACHE = {}


# ---------------------------------------------------------------- wait split
_ctr = [0]


def _mk_nop(engine, waits=None, updates=None):
    _ctr[0] += 1
    si = mybir.SyncInfo(on_wait=waits or [], on_update=updates or [])
    return bass_rust.InstNoOp(
        name=f"I-waitfix-{_ctr[0]}", engine=engine, ins=[], outs=[], sync_info=si
    )


def split_multi_waits(nc):
    """This walrus build accepts only ONE sync wait/update per instruction;
    hoist extras onto adjacent same-engine NoOp carriers."""
    for fn in nc.m.functions:
        for bb in fn.blocks:
            insts = bb.instructions
            out = []
            changed = False
            for inst in insts:
                si = inst.sync_info
                if si is None:
                    out.append(inst)
                    continue
                waits = list(si.on_wait or [])
                updates = list(si.on_update or [])
                pre, post = [], []
                if len(waits) > 1:
                    for w in waits[:-1]:
                        pre.append(_mk_nop(inst.engine, waits=[w]))
                    si.on_wait = [waits[-1]]
                    changed = True
                if len(updates) > 1:
                    if inst.opcode == "DMACopy":
                        raise RuntimeError(
                            f"DMACopy {inst.name} has {len(updates)} updates"
                        )
                    for u in updates[1:]:
                        post.append(_mk_nop(inst.engine, updates=[u]))
                    si.on_update = [updates[0]]
                    changed = True
                out.extend(pre)
                out.append(inst)
                out.extend(post)
            if changed:
                insts[:] = out
    return nc


# ---------------------------------------------------------------- host utils
def round_f32r(a: np.ndarray) -> np.ndarray:
    """Round fp32 -> float32r (RNE to 11 mantissa bits), exact device match."""
    a = np.ascontiguousarray(a, dtype=np.float32)
    b = a.view(np.uint32).astype(np.uint64)
    drop = 12
    half = np.uint64(1 << (drop - 1))
    lsb = (b >> np.uint64(drop)) & np.uint64(1)
    out = (((b + half - np.uint64(1) + lsb) >> np.uint64(drop)) << np.uint64(drop))
    return out.astype(np.uint32).view(np.float32)


def _rope_tables(norm_w: np.ndarray):
    """cosw[n,d] = cos[n,d]*w[d];  sinw folds the rotate-half sign+swap of w:
    q' = qn*cosw + shuffle32(qn)*sinw  (shuffle32 = swap halves, no negation)."""
    inv_freq = 1.0 / (THETA ** (np.arange(0, HD, 2, dtype=np.float32) / HD))
    t = np.arange(N, dtype=np.float32)
    freqs = np.einsum("i,j->ij", t, inv_freq).astype(np.float32)
    emb = np.concatenate([freqs, freqs], axis=-1)  # [N, HD]
    cos = np.cos(emb).astype(np.float32)
    sin = np.sin(emb).astype(np.float32)
    w = norm_w.astype(np.float32)
    cosw = cos * w[None, :]
    sinw = np.empty_like(sin)
    h = HD // 2
    sinw[:, :h] = -sin[:, :h] * w[None, h:]
    sinw[:, h:] = sin[:, h:] * w[None, :h]
    return cosw, sinw


# ---------------------------------------------------------------- bass build
def build_nc(use_bias: bool, shared_tabs: bool, debug: bool = False):
    FC = 7 if use_bias else 6  # feature chunks of 128 (7th = bias row)
    nc = bass.Bass()

    xt_d = nc.dram_tensor("xt", [FC * 128, N], F32R, kind="ExternalInput")
    wq_d = nc.dram_tensor("wq", [FC * 128, HPC * HD], F32R, kind="ExternalInput")
    wk_d = nc.dram_tensor("wk", [FC * 128, HPC * HD], F32R, kind="ExternalInput")
    wv_d = nc.dram_tensor("wv", [FC * 128, HPC * HD], F32R, kind="ExternalInput")
    wo_d = nc.dram_tensor("wo", [HPC * HD, D], F32R, kind="ExternalInput")
    cosq_d = nc.dram_tensor("cosq", [N, HD], F32, kind="ExternalInput")
    sinq_d = nc.dram_tensor("sinq", [N, HD], F32, kind="ExternalInput")
    if shared_tabs:
        cosk_d, sink_d = None, None
    else:
        cosk_d = nc.dram_tensor("cosk", [N, HD], F32, kind="ExternalInput")
        sink_d = nc.dram_tensor("sink", [N, HD], F32, kind="ExternalInput")
    y_d = nc.dram_tensor("y", [N, D], F32, kind="ExternalOutput")
    dbg = {}
    if debug:
        dbg["qt"] = nc.dram_tensor("dbg_qt", [128, 3, N], F32, kind="ExternalOutput")
        dbg["kt"] = nc.dram_tensor("dbg_kt", [128, 3, N], F32, kind="ExternalOutput")
        dbg["v"] = nc.dram_tensor("dbg_v", [128, NT, HPC, 65], F32, kind="ExternalOutput")
        dbg["ot"] = nc.dram_tensor("dbg_ot", [128, 3, N], F32, kind="ExternalOutput")
        dbg["den"] = nc.dram_tensor("dbg_den", [8, 4, 512], F32, kind="ExternalOutput")

    with tile.TileContext(nc) as tc:
        with (
            tc.tile_pool(name="const", bufs=1) as constp,
            tc.tile_pool(name="wts", bufs=1) as wts,
            tc.tile_pool(name="persist", bufs=1) as persist,
            tc.tile_pool(name="rope", bufs=2) as rope,
            tc.tile_pool(name="pt", bufs=2) as ptp,
            tc.tile_pool(name="posta", bufs=2) as posta,
            tc.tile_pool(name="xtile", bufs=3) as xtile,
            tc.tile_pool(name="otq", bufs=2) as otqp,
            tc.tile_pool(name="yout", bufs=2) as yout,
            tc.tile_pool(name="work", bufs=2, space="PSUM") as workp,
            tc.tile_pool(name="otp", bufs=2, space="PSUM") as otp,
        ):
            # ---- constants / weights
            ident = constp.tile([128, 128], F32)
            make_identity(nc, ident[:, :])
            ones64 = constp.tile([128, 64], F32R)
            nc.vector.memset(ones64[:, :].bitcast(F32), 1.0)
            eps_t = constp.tile([128, 1], F32)
            nc.vector.memset(eps_t[:, :], EPS)
            zero_t = constp.tile([128, 1], F32)
            nc.vector.memset(zero_t[:, :], 0.0)


            w_sbs = []
            for wd, nm in ((wq_d, "wq"), (wk_d, "wk"), (wv_d, "wv")):
                wsb = wts.tile([128, FC, HPC * HD], F32R, tag=nm, name=nm)
                nc.sync.dma_start(wsb[:, :, :], wd.rearrange("(c p) n -> p c n", p=128))
                w_sbs.append(wsb)
            wo_sb = wts.tile([128, 3, D], F32R, tag="wo")
            nc.sync.dma_start(wo_sb[:, :, :], wo_d.rearrange("(c p) n -> p c n", p=128))

            tabs = {}
            tab_srcs = [(cosq_d, "cosq"), (sinq_d, "sinq")]
            if not shared_tabs:
                tab_srcs += [(cosk_d, "cosk"), (sink_d, "sink")]
            for td, nm in tab_srcs:
                tsb = constp.tile([128, NT, HD], F32, tag=nm, name=nm)
                nc.sync.dma_start(tsb[:, :, :], td.rearrange("(t p) d -> p t d", p=128))
                tabs[nm] = tsb
            if shared_tabs:
                tabs["cosk"] = tabs["cosq"]
                tabs["sink"] = tabs["sinq"]

            qt_sb = persist.tile([128, 3, N], mybir.dt.bfloat16, tag="qt")
            kt_sb = persist.tile([128, 3, N], mybir.dt.bfloat16, tag="kt")
            vaug = persist.tile([128, NT, HPC, 65], F32R, tag="vaug")
            nc.vector.memset(vaug[:, :, :, 64:65].bitcast(F32), 1.0)


            # ================= phase 1: qkv + norm/rope + transposes
            for i in range(NT):
                xt_sb = xtile.tile([128, FC, 128], F32R, tag="xt")
                nc.sync.dma_start(
                    xt_sb[:, :, :],
                    xt_d.rearrange("(c p) n -> p c n", p=128)[:, :, i * 128:(i + 1) * 128],
                )
                for t in range(3):
                    ps = workp.tile([128, HPC * HD], F32, tag="work")
                    for c in range(FC):
                        nc.tensor.matmul(
                            ps[:, :],
                            xt_sb[:, c, :],
                            w_sbs[t][:, c, :],
                            start=(c == 0),
                            stop=(c == FC - 1),
                        )
                    if t == 2:  # V: straight to augmented layout (f32r)
                        nc.vector.tensor_copy(
                            vaug[:, i, :, 0:64],
                            ps[:, :].rearrange("p (h d) -> p h d", h=HPC),
                        )
                        continue
                    # Q or K: copy to SBUF then rmsnorm+rope
                    qsb = rope.tile([128, HPC, HD], F32, tag="qsb")
                    nc.vector.tensor_copy(qsb[:, :, :], ps[:, :].rearrange("p (h d) -> p h d", h=HPC))

                    sq = rope.tile([128, HPC, HD], F32, tag="sq")
                    nc.vector.tensor_mul(sq[:, :, :], qsb[:, :, :], qsb[:, :, :])
                    ss = rope.tile([128, HPC], F32, tag="ss")
                    nc.vector.reduce_sum(ss[:, :], sq[:, :, :], axis=mybir.AxisListType.X)
                    # rsqrt(mean + eps) = exp(-0.5 * ln(sum/64 + eps))
                    lg = rope.tile([128, HPC], F32, tag="lg")
                    nc.scalar.activation(lg[:, :], ss[:, :],
                                         mybir.ActivationFunctionType.Ln,
                                         bias=eps_t[:, :], scale=1.0 / HD)
                    rs = rope.tile([128, HPC], F32, tag="rs")
                    nc.scalar.activation(rs[:, :], lg[:, :],
                                         mybir.ActivationFunctionType.Exp,
                                         bias=zero_t[:, :], scale=-0.5)

                    cosn = "cosq" if t == 0 else "cosk"
                    sinn = "sinq" if t == 0 else "sink"
                    cosb = tabs[cosn][:, i, None, :].to_broadcast((128, HPC, HD))
                    sinb = tabs[sinn][:, i, None, :]
                    a = rope.tile([128, HPC, HD], F32, tag="a")
                    nc.vector.tensor_mul(a[:, :, :], qsb[:, :, :], cosb)
                    bt = rope.tile([128, HPC, HD], F32, tag="bt")
                    h = HD // 2
                    nc.vector.tensor_mul(
                        bt[:, :, 0:h], qsb[:, :, h:HD],
                        sinb[:, :, 0:h].to_broadcast((128, HPC, h)),
                    )
                    nc.vector.tensor_mul(
                        bt[:, :, h:HD], qsb[:, :, 0:h],
                        sinb[:, :, h:HD].to_broadcast((128, HPC, h)),
                    )
                    c2 = rope.tile([128, HPC, HD], F32, tag="c2")
                    nc.vector.tensor_add(c2[:, :, :], a[:, :, :], bt[:, :, :])
                    ro = rope.tile([128, HPC, HD], F32, tag="ro")
                    nc.vector.tensor_mul(
                        ro[:, :, :], c2[:, :, :],
                        rs[:, :, None].to_broadcast((128, HPC, HD)),
                    )
                    # transpose [128 seq, 384] -> three [128, 128] blocks
                    dst = qt_sb if t == 0 else kt_sb
                    rof = ro.rearrange("p h d -> p (h d)")
                    for blk in range(3):
                        tp = otp.tile([128, 128], F32, tag="ot")
                        nc.tensor.transpose(tp[:, :], rof[:, blk * 128:(blk + 1) * 128],
                                            ident[:, :])
                        nc.vector.tensor_copy(dst[:, blk, i * 128:(i + 1) * 128], tp[:, :])

            if debug:
                qtf = persist.tile([128, 3, N], F32, tag="dbgqt")
                nc.vector.tensor_copy(qtf[:, :, :], qt_sb[:, :, :])
                nc.sync.dma_start(dbg["qt"][:, :, :], qtf[:, :, :])
                ktf = persist.tile([128, 3, N], F32, tag="dbgkt")
                nc.vector.tensor_copy(ktf[:, :, :], kt_sb[:, :, :])
                nc.sync.dma_start(dbg["kt"][:, :, :], ktf[:, :, :])
                nc.sync.dma_start(dbg["v"][:, :, :, :], vaug[:, :, :, :].bitcast(F32))

            # ================= phase 2: attention per (qc, head)
            GROUPS = [(0, 3), (3, 3), (6, 3), (9, 3), (12, 3), (15, 1)]
            for qc in range(4):
                qs = qc * 512
                denA = posta.tile([128, 512], F32, tag="denA")
                denB = posta.tile([128, 512], F32, tag="denB")
                nc.vector.memset(denA[:, :], 1.0)
                nc.vector.memset(denB[:, :], 1.0)
                otun = posta.tile([64, HPC, 512], F32, tag="otun", bufs=1)
                otq = otqp.tile([128, 3, 512], F32R, tag="otq")
                for pp in range(3):
                    ots = []
                    for hh in range(2):
                        otps = otp.tile([65, 512], F32, tag="ot", name=f"otps{hh}")
                        ots.append(otps)
                    for (g0, gsz) in GROUPS:
                        slabs = []
                        for hh in range(2):
                            slab = workp.tile([128, 3, 512], F32, tag="work", name=f"slab{hh}")
                            slabs.append(slab)
                        for hh in range(2):
                            r0, r1 = hh * 64, hh * 64 + 64
                            for gi in range(gsz):
                                kt = g0 + gi
                                nc.tensor.matmul(
                                    slabs[hh][:, gi, :],
                                    kt_sb[r0:r1, pp, kt * 128:(kt + 1) * 128],
                                    qt_sb[r0:r1, pp, qs:qs + 512],
                                    start=True, stop=True,
                                )
                        for hh in range(2):
                            pt = ptp.tile([128, 3, 512], F32R, tag="pt")
                            nc.scalar.activation(
                                pt[:, 0:gsz, :], slabs[hh][:, 0:gsz, :],
                                mybir.ActivationFunctionType.Exp,
                                bias=zero_t[:, :], scale=SCALE,
                            )
                            hloc = pp * 2 + hh
                            for gi in range(gsz):
                                kt = g0 + gi
                                nc.tensor.matmul(
                                    ots[hh][:, :],
                                    vaug[:, kt, hloc, :],
                                    pt[:, gi, :],
                                    start=(kt == 0), stop=(kt == NT - 1),
                                )
                    for hh in range(2):
                        hloc = pp * 2 + hh
                        dt_, dr = (denA, hloc) if hloc < 4 else (denB, hloc - 4)
                        nc.vector.tensor_copy(dt_[32 * dr:32 * dr + 1, :], ots[hh][64:65, :])
                        nc.vector.tensor_copy(otun[:, hloc, :], ots[hh][0:64, :])

                recA = posta.tile([128, 512], F32R, tag="recA")
                recB = posta.tile([128, 512], F32R, tag="recB")
                with nc.allow_low_precision(reason="f32r recip for PE broadcast"):
                    nc.vector.reciprocal(recA[:, :], denA[:, :])
                    nc.vector.reciprocal(recB[:, :], denB[:, :])
                if debug:
                    nc.sync.dma_start(dbg["den"][:, qc, :],
                                      denA[:, :].rearrange("(a b) n -> a b n", b=16)[:, 0, :])
                for hloc in range(HPC):
                    rt, dr = (recA, hloc) if hloc < 4 else (recB, hloc - 4)
                    bc = otp.tile([64, 512], F32, tag="ot", name="bc")
                    nc.tensor.matmul(bc[:, :], ones64[32 * dr:32 * dr + 1, :],
                                     rt[32 * dr:32 * dr + 1, :],
                                     start=True, stop=True,
                                     tile_position=(32 * dr, 0))
                    pp, hh = hloc // 2, hloc % 2
                    nc.vector.tensor_mul(
                        otq[hh * 64:(hh + 1) * 64, pp, :],
                        otun[:, hloc, :],
                        bc[:, :],
                    )
                # ---- projection for this qc
                for qt4 in range(4):
                    q0 = qs + qt4 * 128
                    yps = workp.tile([128, 3, 512], F32, tag="work")
                    for c in range(3):
                        nc.tensor.matmul(
                            yps[:, 0, :], otq[:, c, qt4 * 128:(qt4 + 1) * 128],
                            wo_sb[:, c, 0:512],
                            start=(c == 0), stop=(c == 2),
                        )
                    for c in range(3):
                        nc.tensor.matmul(
                            yps[:, 1, 0:256], otq[:, c, qt4 * 128:(qt4 + 1) * 128],
                            wo_sb[:, c, 512:768],
                            start=(c == 0), stop=(c == 2),
                        )
                    ysb = yout.tile([128, D], F32, tag="ysb")
                    nc.vector.tensor_copy(ysb[:, 0:512], yps[:, 0, :])
                    nc.vector.tensor_copy(ysb[:, 512:768], yps[:, 1, 0:256])
                    nc.sync.dma_start(y_d[q0:q0 + 128, :], ysb[:, :])


    split_multi_waits(nc)
    return nc


# ---------------------------------------------------------------- entry
def kernel(x, qkv_w, qkv_b, proj_w, proj_b, q_norm_w, k_norm_w, _debug=False,
           _trace=False):
    x = np.asarray(x, dtype=np.float32)
    qkv_w = np.asarray(qkv_w, dtype=np.float32)
    qkv_b = np.asarray(qkv_b, dtype=np.float32)
    proj_w = np.asarray(proj_w, dtype=np.float32)
    proj_b = np.asarray(proj_b, dtype=np.float32)
    q_norm_w = np.asarray(q_norm_w, dtype=np.float32)
    k_norm_w = np.asarray(k_norm_w, dtype=np.float32)

    use_bias = bool(np.any(qkv_b != 0.0))
    shared_tabs = bool(np.array_equal(q_norm_w, k_norm_w))
    key = (use_bias, shared_tabs, _debug)
    if key not in _CACHE:
        _CACHE[key] = build_nc(use_bias, shared_tabs, debug=_debug)
    nc = _CACHE[key]
    FC = 7 if use_bias else 6

    cosq, sinq = _rope_tables(q_norm_w)
    cosk, sink = _rope_tables(k_norm_w)

    in_maps = []
    for core in range(8):
        b, hg = core // 2, core % 2
        h0 = hg * HPC
        cols = slice(h0 * HD, (h0 + HPC) * HD)
        xt = np.ascontiguousarray(x[b].T)                       # [768, N]
        wq = qkv_w[:, cols]
        wk = qkv_w[:, D:][:, cols]
        wv = qkv_w[:, 2 * D:][:, cols]
        if use_bias:
            pad = np.zeros((128, N), np.float32)
            pad[0, :] = 1.0
            xt = np.concatenate([xt, pad], axis=0)
            wpad = np.zeros((128, HPC * HD), np.float32)
            wqb = np.concatenate([wq, wpad], axis=0)
            wkb = np.concatenate([wk, wpad], axis=0)
            wvb = np.concatenate([wv, wpad], axis=0)
            wqb[D, :] = qkv_b[cols]
            wkb[D, :] = qkv_b[D:][cols]
            wvb[D, :] = qkv_b[2 * D:][cols]
            wq, wk, wv = wqb, wkb, wvb
        wo = proj_w[h0 * HD:(h0 + HPC) * HD, :]
        im = {
            "xt": round_f32r(xt),
            "wq": round_f32r(wq), "wk": round_f32r(wk), "wv": round_f32r(wv),
            "wo": round_f32r(wo),
            "cosq": cosq, "sinq": sinq,
        }
        if not shared_tabs:
            im["cosk"] = cosk
            im["sink"] = sink
        in_maps.append(im)

    res = run_bass_kernel_spmd(nc, in_maps, core_ids=list(range(8)),
                               trace=_trace or KERNEL_TRACE)
    kernel._last = res

    y = np.empty((B, N, D), dtype=np.float32)
    for b in range(B):
        y[b] = res.results[2 * b]["y"] + res.results[2 * b + 1]["y"] + proj_b[None, :]
    return y



# revision 4
# speedup vs baseline: 1.3486x; 1.3486x over previous
"""Multi-head attention (RMSNorm-QK + RoPE) Trainium2 Bass kernel — v2.

Sharding: 8 cores = 4 batches x 2 head-groups (6 heads each).
Host sums the two partial y's per batch and adds proj bias.

v2 design (vs baseline):
  - qkv GEMM in bf16 (same PE rate, FWL weight loads, half DMA).
  - RMSNorm commutes with RoPE (rotation preserves pair norms): rope runs on
    raw q/k, the rsqrt factor multiplies afterwards, q & k fused per tile.
  - Q/K transposes via DMA xbar (dma_start_transpose) instead of PE+DVE.
  - Phase 2 is one software-pipelined stream over all (qc, pp) chunks:
    QK(c+1) is emitted before PV(c) so the scalar-engine exp stream never
    stalls; PSUM: 2x3-bank QK slabs + 2x1-bank PV accumulators.
  - Per-chunk exp = one ACTIVATE over [128, 3, 512] PSUM.
  - Softmax denominators ride as the 65th V column; reciprocal runs on a
    [128, 24] repacked tile (DMA bounce) instead of [1, 512] strips.
  - Projection/broadcast work for chunk qc is interleaved into qc+1's
    attention stream in small pieces to keep the exp stream saturated.
"""

import sys

for _p in ("/opt/trn_rl_repo", "/root/.axon_site/_ro/trn_rl_repo"):
    if _p not in sys.path:
        sys.path.insert(0, _p)

import numpy as np
import ml_dtypes

import bass_rust
import concourse.bass as bass
import concourse.mybir as mybir
import concourse.tile as tile
from concourse.bass_utils import run_bass_kernel_spmd

# Problem constants (hardcoded per contract)
B, N, D = 4, 2048, 768
H, HD = 12, 64
HPC = 6              # heads per core
NT = N // 128        # 16 seq tiles
EPS = 1e-6
THETA = 10000.0
SCALE = HD ** -0.5   # 0.125

F32 = mybir.dt.float32
F32R = mybir.dt.float32r
BF16 = mybir.dt.bfloat16

KERNEL_TRACE = False
_CACHE = {}


# ---------------------------------------------------------------- wait split
_ctr = [0]


def _mk_nop(engine, waits=None, updates=None):
    _ctr[0] += 1
    si = mybir.SyncInfo(on_wait=waits or [], on_update=updates or [])
    return bass_rust.InstNoOp(
        name=f"I-waitfix-{_ctr[0]}", engine=engine, ins=[], outs=[], sync_info=si
    )


def split_multi_waits(nc):
    """This walrus build accepts only ONE sync wait/update per instruction;
    hoist extras onto adjacent same-engine NoOp carriers."""
    for fn in nc.m.functions:
        for bb in fn.blocks:
            insts = bb.instructions
            out = []
            changed = False
            for inst in insts:
                si = inst.sync_info
                if si is None:
                    out.append(inst)
                    continue
                waits = list(si.on_wait or [])
                updates = list(si.on_update or [])
                pre, post = [], []
                if len(waits) > 1:
                    for w in waits[:-1]:
                        pre.append(_mk_nop(inst.engine, waits=[w]))
                    si.on_wait = [waits[-1]]
                    changed = True
                if len(updates) > 1:
                    if inst.opcode == "DMACopy":
                        raise RuntimeError(
                            f"DMACopy {inst.name} has {len(updates)} updates"
                        )
                    for u in updates[1:]:
                        post.append(_mk_nop(inst.engine, updates=[u]))
                    si.on_update = [updates[0]]
                    changed = True
                out.extend(pre)
                out.append(inst)
                out.extend(post)
            if changed:
                insts[:] = out
    return nc


# ---------------------------------------------------------------- host utils
def _rope_tables(norm_w: np.ndarray):
    """cosw[n,d] = cos[n,d]*w[d];  sinw folds the rotate-half sign+swap of w:
    q' = qn*cosw + shuffle32(qn)*sinw  (shuffle32 = swap halves, no negation)."""
    inv_freq = 1.0 / (THETA ** (np.arange(0, HD, 2, dtype=np.float32) / HD))
    t = np.arange(N, dtype=np.float32)
    freqs = np.einsum("i,j->ij", t, inv_freq).astype(np.float32)
    emb = np.concatenate([freqs, freqs], axis=-1)  # [N, HD]
    cos = np.cos(emb).astype(np.float32)
    sin = np.sin(emb).astype(np.float32)
    w = norm_w.astype(np.float32)
    cosw = cos * w[None, :]
    sinw = np.empty_like(sin)
    h = HD // 2
    sinw[:, :h] = -sin[:, :h] * w[None, h:]
    sinw[:, h:] = sin[:, h:] * w[None, :h]
    return cosw, sinw


# ---------------------------------------------------------------- bass build
def build_nc(use_bias: bool, use_3d_transpose: bool):
    FC = 7 if use_bias else 6  # feature chunks of 128 (7th = bias row)
    nc = bass.Bass()

    xt_d = nc.dram_tensor("xt", [FC * 128, N], BF16, kind="ExternalInput")
    wq_d = nc.dram_tensor("wq", [FC * 128, HPC * HD], BF16, kind="ExternalInput")
    wk_d = nc.dram_tensor("wk", [FC * 128, HPC * HD], BF16, kind="ExternalInput")
    wv_d = nc.dram_tensor("wv", [FC * 128, HPC * HD], BF16, kind="ExternalInput")
    wo_d = nc.dram_tensor("wo", [HPC * HD, D], BF16, kind="ExternalInput")
    # rope tables: [:, 0, :] = q variant, [:, 1, :] = k variant (norm_w folded)
    cos_d = nc.dram_tensor("cost", [N, 2 * HD], F32, kind="ExternalInput")
    sin_d = nc.dram_tensor("sint", [N, 2 * HD], F32, kind="ExternalInput")
    y_d = nc.dram_tensor("y", [N, D], F32, kind="ExternalOutput")
    den_dram = nc.dram_tensor("den_scr", [4, HPC, 512], F32, kind="Internal")
    rec_dram = nc.dram_tensor("rec_scr", [4, HPC, 512], F32R, kind="Internal")

    with tile.TileContext(nc) as tc:
        with (
            tc.tile_pool(name="const", bufs=1) as constp,
            tc.tile_pool(name="wts", bufs=1) as wts,
            tc.tile_pool(name="persist", bufs=1) as persist,
            tc.tile_pool(name="rope", bufs=2) as rope,
            tc.tile_pool(name="xtile", bufs=3) as xtile,
            tc.tile_pool(name="ptp", bufs=3) as ptp,
            tc.tile_pool(name="otunp", bufs=2) as otunp,
            tc.tile_pool(name="otqp", bufs=2) as otqp,
            tc.tile_pool(name="denp", bufs=2) as denp,
            tc.tile_pool(name="yout", bufs=2) as yout,
            tc.tile_pool(name="work", bufs=2, space="PSUM") as workp,
            tc.tile_pool(name="otp", bufs=2, space="PSUM") as otp,
        ):
            # ---- constants / weights
            ones_sb = constp.tile([128, 64], F32R)
            nc.vector.memset(ones_sb[:, :].bitcast(F32), 1.0)
            eps_t = constp.tile([128, 1], F32)
            nc.vector.memset(eps_t[:, :], EPS)
            zero_t = constp.tile([128, 1], F32)
            nc.vector.memset(zero_t[:, :], 0.0)

            w_sbs = []
            for wd, nm in ((wq_d, "wq"), (wk_d, "wk"), (wv_d, "wv")):
                wsb = wts.tile([128, FC, HPC * HD], BF16, tag=nm, name=nm)
                nc.sync.dma_start(wsb[:, :, :], wd.rearrange("(c p) n -> p c n", p=128))
                w_sbs.append(wsb)
            wo_sb = wts.tile([128, 3, D], BF16, tag="wo")
            nc.sync.dma_start(wo_sb[:, :, :], wo_d.rearrange("(c p) n -> p c n", p=128))

            # rope tables, cast f32 -> bf16 during DMA (SWDGE)
            cos_sb = constp.tile([128, NT, 2, HD], BF16, tag="cos")
            nc.gpsimd.dma_start(
                cos_sb[:, :, :, :],
                cos_d.rearrange("(t p) (a d) -> p t a d", p=128, a=2),
            )
            sin_sb = constp.tile([128, NT, 2, HD], BF16, tag="sin")
            nc.gpsimd.dma_start(
                sin_sb[:, :, :, :],
                sin_d.rearrange("(t p) (a d) -> p t a d", p=128, a=2),
            )

            qt_sb = persist.tile([128, 3, N], BF16, tag="qt")
            kt_sb = persist.tile([128, 3, N], BF16, tag="kt")
            vaug = persist.tile([128, NT, HPC, 65], BF16, tag="vaug")
            nc.vector.memset(vaug[:, :, :, 64:65], 1.0)

            # ================= phase 1: qkv + (rope, then rmsnorm-scale) + T
            for i in range(NT):
                xt_sb = xtile.tile([128, FC, 128], BF16, tag="xt")
                nc.sync.dma_start(
                    xt_sb[:, :, :],
                    xt_d.rearrange("(c p) n -> p c n", p=128)[:, :, i * 128:(i + 1) * 128],
                )
                ps = workp.tile([128, 3, 512], F32, tag="work")
                for c in range(FC):
                    for t in range(3):
                        nc.tensor.matmul(
                            ps[:, t, 0:384],
                            xt_sb[:, c, :],
                            w_sbs[t][:, c, :],
                            start=(c == 0),
                            stop=(c == FC - 1),
                        )
                # V -> vaug (scalar engine, PSUM->SBUF cast copy)
                nc.scalar.copy(
                    vaug[:, i, :, 0:64],
                    ps[:, 2, 0:384].rearrange("p (h d) -> p h d", h=HPC),
                )
                # q|k fused [128, 2, 6, 64] bf16
                qk = rope.tile([128, 2, HPC, HD], BF16, tag="qk")
                nc.scalar.copy(
                    qk[:, :, :, :],
                    ps[:, 0:2, 0:384].rearrange("p a (h d) -> p a h d", h=HPC),
                )
                sq = rope.tile([128, 2, HPC, HD], BF16, tag="sq")
                nc.scalar.activation(
                    sq[:, :, :, :],
                    ps[:, 0:2, 0:384].rearrange("p a (h d) -> p a h d", h=HPC),
                    mybir.ActivationFunctionType.Square,
                    bias=zero_t[:, :],
                )
                ss = rope.tile([128, 2, HPC], F32, tag="ss")
                nc.vector.reduce_sum(ss[:, :, :], sq[:, :, :, :], axis=mybir.AxisListType.X)
                # rsqrt(mean + eps) = exp(-0.5 * ln(sum/64 + eps))
                lg = rope.tile([128, 2, HPC], F32, tag="lg")
                nc.scalar.activation(lg[:, :, :], ss[:, :, :],
                                     mybir.ActivationFunctionType.Ln,
                                     bias=eps_t[:, :], scale=1.0 / HD)
                rs = rope.tile([128, 2, HPC], BF16, tag="rs")
                nc.scalar.activation(rs[:, :, :], lg[:, :, :],
                                     mybir.ActivationFunctionType.Exp,
                                     bias=zero_t[:, :], scale=-0.5)

                cosb = cos_sb[:, i, :, None, :].to_broadcast((128, 2, HPC, HD))
                sinb = sin_sb[:, i, :, None, :]
                a = rope.tile([128, 2, HPC, HD], BF16, tag="a")
                nc.vector.tensor_mul(a[:, :, :, :], qk[:, :, :, :], cosb)
                bt = rope.tile([128, 2, HPC, HD], BF16, tag="bt")
                h = HD // 2
                nc.vector.tensor_mul(
                    bt[:, :, :, 0:h], qk[:, :, :, h:HD],
                    sinb[:, :, :, 0:h].to_broadcast((128, 2, HPC, h)),
                )
                nc.vector.tensor_mul(
                    bt[:, :, :, h:HD], qk[:, :, :, 0:h],
                    sinb[:, :, :, h:HD].to_broadcast((128, 2, HPC, h)),
                )
                c2 = rope.tile([128, 2, HPC, HD], BF16, tag="c2")
                nc.vector.tensor_add(c2[:, :, :, :], a[:, :, :, :], bt[:, :, :, :])
                ro = rope.tile([128, 2, HPC, HD], BF16, tag="ro")
                nc.vector.tensor_mul(
                    ro[:, :, :, :], c2[:, :, :, :],
                    rs[:, :, :, None].to_broadcast((128, 2, HPC, HD)),
                )
                rof = ro.rearrange("p a h d -> p (a h d)")
                if use_3d_transpose:
                    nc.sync.dma_start_transpose(
                        qt_sb[:, 0:3, i * 128:(i + 1) * 128], rof[:, 0:384])
                    nc.sync.dma_start_transpose(
                        kt_sb[:, 0:3, i * 128:(i + 1) * 128], rof[:, 384:768])
                else:
                    for blk in range(3):
                        nc.sync.dma_start_transpose(
                            qt_sb[:, blk, i * 128:(i + 1) * 128],
                            rof[:, blk * 128:(blk + 1) * 128])
                        nc.sync.dma_start_transpose(
                            kt_sb[:, blk, i * 128:(i + 1) * 128],
                            rof[:, 384 + blk * 128:384 + (blk + 1) * 128])

            # ================= phase 2: one pipelined stream over (qc, pp)
            units = []
            for qc in range(4):
                for pp in range(3):
                    fills = [(kt, hh) for kt in range(NT) for hh in range(2)]
                    chunks = [fills[j:j + 3] for j in range(0, 32, 3)]
                    for ci, ch in enumerate(chunks):
                        units.append((qc, pp, ch, ci == len(chunks) - 1))

            cur_ots = {}
            otun_by_qc = {}
            rec_by_qc = {}
            finish_pieces = []  # queue of callables, popped one per unit
            pend = None         # (qc, pp, ch, pt, last)

            def emit_den_chain(qc):
                otun_all = otun_by_qc[qc]
                # den rows -> DRAM -> [128, 24] repack -> recip -> back
                nc.sync.dma_start(den_dram[qc, :, :], otun_all[64:65, :, :])
                den_t = denp.tile([128, HPC, 4], F32, tag="dent")
                nc.sync.dma_start(
                    den_t[:, :, :],
                    den_dram[qc].rearrange("h (c p) -> p h c", p=128),
                )
                rec_t = denp.tile([128, HPC, 4], F32R, tag="rect")
                with nc.allow_low_precision(reason="f32r recip for PE broadcast"):
                    nc.vector.reciprocal(rec_t[:, :, :], den_t[:, :, :])
                nc.sync.dma_start(
                    rec_dram[qc].rearrange("h (c p) -> p h c", p=128),
                    rec_t[:, :, :],
                )
                rec_sb = denp.tile([1, HPC, 512], F32R, tag="rec")
                nc.sync.dma_start(rec_sb[:, :, :], rec_dram[qc, None, :, :])
                rec_by_qc[qc] = rec_sb

            def emit_finish(qc):
                """Queue bc+normalize and projection for a completed qc."""
                otun_all = otun_by_qc[qc]
                rec_sb = rec_by_qc[qc]
                otq = otqp.tile([128, 3, 512], BF16, tag="otq", name=f"otq{qc}")

                def norm_piece(h0, h1):
                    def fn():
                        for hloc in range(h0, h1):
                            pp_, hh_ = hloc // 2, hloc % 2
                            bc = otp.tile([128, 512], F32, tag="ot", name="bc")
                            nc.tensor.matmul(bc[0:64, :], ones_sb[0:1, :],
                                             rec_sb[0:1, hloc, :],
                                             start=True, stop=True)
                            nc.vector.tensor_mul(
                                otq[hh_ * 64:(hh_ + 1) * 64, pp_, :],
                                otun_all[0:64, hloc, :],
                                bc[0:64, :],
                            )
                    return fn

                def proj_piece(t0, t1):
                    def fn():
                        for qt4 in range(t0, t1):
                            q0 = qc * 512 + qt4 * 128
                            yps = workp.tile([128, 3, 512], F32, tag="work",
                                             name="yps")
                            for c in range(3):
                                nc.tensor.matmul(
                                    yps[:, 0, :],
                                    otq[:, c, qt4 * 128:(qt4 + 1) * 128],
                                    wo_sb[:, c, 0:512],
                                    start=(c == 0), stop=(c == 2),
                                )
                            for c in range(3):
                                nc.tensor.matmul(
                                    yps[:, 1, 0:256],
                                    otq[:, c, qt4 * 128:(qt4 + 1) * 128],
                                    wo_sb[:, c, 512:768],
                                    start=(c == 0), stop=(c == 2),
                                )
                            ysb = yout.tile([128, D], F32, tag="ysb")
                            nc.vector.tensor_copy(ysb[:, 0:512], yps[:, 0, :])
                            nc.vector.tensor_copy(ysb[:, 512:768], yps[:, 1, 0:256])
                            nc.sync.dma_start(y_d[q0:q0 + 128, :], ysb[:, :])
                    return fn

                # All bc allocations (otp pool) must land before the next
                # (qc, pp)'s PV accumulators claim the otp slots, else the
                # slot-acquisition waits on a release that sits later in the
                # PE queue (deadlock). So norm work is one immediate piece.
                norm_piece(0, 6)()
                finish_pieces.append(proj_piece(0, 2))
                finish_pieces.append(proj_piece(2, 4))

            def flush_pv(p):
                qc, pp, ch, pt, last = p
                key = (qc, pp)
                if key not in cur_ots:
                    cur_ots[key] = [
                        otp.tile([128, 512], F32, tag="ot", name=f"ots{hh}")
                        for hh in range(2)
                    ]
                ots = cur_ots[key]
                for j, (kt, hh) in enumerate(ch):
                    nc.tensor.matmul(
                        ots[hh][0:65, :],
                        vaug[:, kt, pp * 2 + hh, :],
                        pt[:, j, :],
                        start=(kt == 0), stop=(kt == NT - 1),
                    )
                if last:
                    if qc not in otun_by_qc:
                        otun_by_qc[qc] = otunp.tile(
                            [65, HPC, 512], F32, tag="otun", name=f"otun{qc}")
                    otun_all = otun_by_qc[qc]
                    for hh in range(2):
                        nc.vector.tensor_copy(
                            otun_all[0:65, pp * 2 + hh, :], ots[hh][0:65, :])
                    del cur_ots[key]
                    if pp == 2:
                        emit_den_chain(qc)
                    if pp == 0 and qc > 0:
                        emit_finish(qc - 1)

            for (qc, pp, ch, last) in units:
                slab = workp.tile([128, 3, 512], F32, tag="work", name="slab")
                for j, (kt, hh) in enumerate(ch):
                    nc.tensor.matmul(
                        slab[:, j, :],
                        kt_sb[hh * 64:(hh + 1) * 64, pp, kt * 128:(kt + 1) * 128],
                        qt_sb[hh * 64:(hh + 1) * 64, pp, qc * 512:(qc + 1) * 512],
                        start=True, stop=True,
                    )
                pt = ptp.tile([128, 3, 512], BF16, tag="pt")
                nj = len(ch)
                nc.scalar.activation(
                    pt[:, 0:nj, :], slab[:, 0:nj, :],
                    mybir.ActivationFunctionType.Exp,
                    bias=zero_t[:, :], scale=SCALE,
                )
                if pend is not None:
                    flush_pv(pend)
                    if finish_pieces:
                        finish_pieces.pop(0)()
                pend = (qc, pp, ch, pt, last)
            flush_pv(pend)
            emit_finish(3)
            while finish_pieces:
                finish_pieces.pop(0)()

    split_multi_waits(nc)
    return nc


# ---------------------------------------------------------------- entry
def kernel(x, qkv_w, qkv_b, proj_w, proj_b, q_norm_w, k_norm_w, _trace=False,
           _debug=False):
    x = np.asarray(x, dtype=np.float32)
    qkv_w = np.asarray(qkv_w, dtype=np.float32)
    qkv_b = np.asarray(qkv_b, dtype=np.float32)
    proj_w = np.asarray(proj_w, dtype=np.float32)
    proj_b = np.asarray(proj_b, dtype=np.float32)
    q_norm_w = np.asarray(q_norm_w, dtype=np.float32)
    k_norm_w = np.asarray(k_norm_w, dtype=np.float32)

    use_bias = bool(np.any(qkv_b != 0.0))
    key = (use_bias, True)
    if key not in _CACHE:
        _CACHE[key] = build_nc(use_bias, use_3d_transpose=True)
    nc = _CACHE[key]
    FC = 7 if use_bias else 6

    cosq, sinq = _rope_tables(q_norm_w)
    cosk, sink = _rope_tables(k_norm_w)
    cost = np.concatenate([cosq, cosk], axis=1)  # [N, 128]
    sint = np.concatenate([sinq, sink], axis=1)

    bf16 = ml_dtypes.bfloat16
    in_maps = []
    for core in range(8):
        b, hg = core // 2, core % 2
        h0 = hg * HPC
        cols = slice(h0 * HD, (h0 + HPC) * HD)
        xt = np.ascontiguousarray(x[b].T)                       # [768, N]
        wq = qkv_w[:, cols]
        wk = qkv_w[:, D:][:, cols]
        wv = qkv_w[:, 2 * D:][:, cols]
        if use_bias:
            pad = np.zeros((128, N), np.float32)
            pad[0, :] = 1.0
            xt = np.concatenate([xt, pad], axis=0)
            wpad = np.zeros((128, HPC * HD), np.float32)
            wqb = np.concatenate([wq, wpad], axis=0)
            wkb = np.concatenate([wk, wpad], axis=0)
            wvb = np.concatenate([wv, wpad], axis=0)
            wqb[D, :] = qkv_b[cols]
            wkb[D, :] = qkv_b[D:][cols]
            wvb[D, :] = qkv_b[2 * D:][cols]
            wq, wk, wv = wqb, wkb, wvb
        wo = proj_w[h0 * HD:(h0 + HPC) * HD, :]
        im = {
            "xt": xt.astype(bf16),
            "wq": np.ascontiguousarray(wq).astype(bf16),
            "wk": np.ascontiguousarray(wk).astype(bf16),
            "wv": np.ascontiguousarray(wv).astype(bf16),
            "wo": np.ascontiguousarray(wo).astype(bf16),
            "cost": cost, "sint": sint,
        }
        in_maps.append(im)

    res = run_bass_kernel_spmd(nc, in_maps, core_ids=list(range(8)),
                               trace=_trace or KERNEL_TRACE)
    kernel._last = res

    y = np.empty((B, N, D), dtype=np.float32)
    for b in range(B):
        y[b] = res.results[2 * b]["y"] + res.results[2 * b + 1]["y"] + proj_b[None, :]
    return y
